# revision 4
# baseline (speedup 1.0000x reference)
"""Multi-head attention (B=2, H=16, S=2048, Dh=64) on 8 trn2 NeuronCores.

Sharding: core c handles batch c//4, heads (c%4)*4 .. +4. Each core computes
attention for its 4 (b,h) pairs independently (no collectives).

Device algorithm per (head, q-chunk of 512):
  S^T[k,q] = K @ Q^T   (bf16 matmuls, k-tile pairs row-packed into the
                        128-deep PE array since the contraction dim is 64)
  attE     = exp(0.125 * S^T)          (ScalarE, reads PSUM, writes bf16 SBUF)
  att      = attE * mask^T             (VectorE bf16; mask==0 rows -> 0)
  O^T[d,q] += [V | 1]^T @ att          (accumulated over k in PSUM; the ones
                                        column makes row 64 the softmax sums)
Host: normalize O^T rows by the sums row and transpose back to [S, Dh].
"""

import numpy as np
import ml_dtypes

TRACE = False
LAST = {}

B, H, S, Dh = 2, 16, 2048, 64
HPC = 4            # heads per core
NCORES = 8
SCALE = 0.125      # Dh ** -0.5
KT = 16            # k tiles of 128
KTP = 8            # k tile pairs
QC = 4             # q chunks of 512
BF16 = ml_dtypes.bfloat16

_NC = None


def _split_waits(nc, max_waits=1):
    """This container's walrus rejects >max_waits semaphore waits on one
    instruction (CoreV3 setupSyncWait "Too many sync wait commands"). Move
    the excess onto NoOps inserted just before, on the same engine — the
    per-engine instruction stream order is preserved, so the waits still
    complete before the original instruction issues."""
    import concourse.mybir as mybir

    ctr = 0
    for f in nc.m.functions:
        for b in f.blocks:
            insts = b.instructions
            new = []
            for inst in insts:
                si = inst.sync_info
                waits = list(si.on_wait) if si else []
                if len(waits) > max_waits:
                    for w in waits[:-max_waits]:
                        ctr += 1
                        new.append(
                            mybir.InstNoOp(
                                name=f"waitsplit-{ctr}",
                                engine=inst.engine,
                                ins=[],
                                outs=[],
                                sync_info=mybir.SyncInfo(on_wait=[w], on_update=[]),
                            )
                        )
                    inst.sync_info = mybir.SyncInfo(
                        on_wait=waits[-max_waits:], on_update=list(si.on_update)
                    )
                new.append(inst)
            insts[:] = new


def _build_nc():
    import concourse.bass as bass
    import concourse.mybir as mybir
    import concourse.tile as tile

    bf = mybir.dt.bfloat16
    f32 = mybir.dt.float32
    Exp = mybir.ActivationFunctionType.Exp

    nc = bass.Bass()
    qT2_e = nc.declare_dram_parameter("qT2", [HPC, 128, S], bf, isOutput=False)
    kT2_e = nc.declare_dram_parameter("kT2", [HPC, 128, KTP * 128], bf, isOutput=False)
    vh_e = nc.declare_dram_parameter("vh", [HPC, 128, KT * 65], bf, isOutput=False)
    mF_e = nc.declare_dram_parameter("maskF", [KTP, 128, QC * 1024], bf, isOutput=False)
    out_e = nc.declare_dram_parameter("out", [HPC, 65, S], f32, isOutput=True)

    with tile.TileContext(nc) as tc:
        with (
            tc.tile_pool(name="maskp", bufs=KTP) as maskp,
            tc.tile_pool(name="qp", bufs=2) as qp,
            tc.tile_pool(name="kp", bufs=2) as kp,
            tc.tile_pool(name="vp", bufs=2) as vp,
            tc.tile_pool(name="attp", bufs=3) as attp,
            tc.tile_pool(name="obp", bufs=2) as obp,
            tc.tile_pool(name="psp", bufs=3, space=bass.MemorySpace.PSUM) as psp,
            tc.tile_pool(name="pop", bufs=2, space=bass.MemorySpace.PSUM) as pop,
        ):
            mtiles = []
            for tp in range(KTP):
                mt = maskp.tile([128, QC * 1024], bf)
                nc.sync.dma_start(mt[:], mF_e[tp])
                mtiles.append(mt)

            for h in range(HPC):
                qt = qp.tile([128, S], bf)
                nc.sync.dma_start(qt[:], qT2_e[h])
                kt_ = kp.tile([128, KTP * 128], bf)
                nc.sync.dma_start(kt_[:], kT2_e[h])
                vt = vp.tile([128, KT * 65], bf)
                nc.sync.dma_start(vt[:], vh_e[h])

                for qq in range(QC):
                    o_ps = pop.tile([65, 512], f32)
                    for tp in range(KTP):
                        s_ps = psp.tile([128, 1024], f32)
                        # S^T tiles for k-tiles 2tp (cols 0:512) and 2tp+1
                        # (cols 512:1024), row-packed in the PE array
                        nc.tensor.matmul(
                            s_ps[:, 0:512],
                            kt_[0:64, tp * 128 : (tp + 1) * 128],
                            qt[0:64, qq * 512 : (qq + 1) * 512],
                            start=True,
                            stop=True,
                        )
                        nc.tensor.matmul(
                            s_ps[:, 512:1024],
                            kt_[64:128, tp * 128 : (tp + 1) * 128],
                            qt[64:128, qq * 512 : (qq + 1) * 512],
                            start=True,
                            stop=True,
                        )
                        attE = attp.tile([128, 1024], bf, tag="attE")
                        nc.scalar.activation(attE[:], s_ps[:], Exp, scale=SCALE)
                        att = attp.tile([128, 1024], bf, tag="attm")
                        nc.vector.tensor_mul(
                            att[:],
                            attE[:],
                            mtiles[tp][:, qq * 1024 : (qq + 1) * 1024],
                        )
                        nc.tensor.matmul(
                            o_ps[:],
                            vt[:, (2 * tp) * 65 : (2 * tp + 1) * 65],
                            att[:, 0:512],
                            start=(tp == 0),
                            stop=False,
                        )
                        nc.tensor.matmul(
                            o_ps[:],
                            vt[:, (2 * tp + 1) * 65 : (2 * tp + 2) * 65],
                            att[:, 512:1024],
                            start=False,
                            stop=(tp == KTP - 1),
                        )
                    o_sb = obp.tile([65, 512], f32)
                    nc.vector.tensor_copy(o_sb[:], o_ps[:])
                    nc.sync.dma_start(
                        out_e[h, :, qq * 512 : (qq + 1) * 512], o_sb[:]
                    )
    _split_waits(nc)
    return nc


def _core_inputs(q, k, v, mask, core):
    b = core // HPC
    h0 = (core % HPC) * HPC
    qh = q[b, h0 : h0 + HPC].transpose(0, 2, 1)          # [4, 64, S]
    qT2 = np.concatenate([qh, qh], axis=1)               # [4, 128, S]
    kh = k[b, h0 : h0 + HPC].transpose(0, 2, 1)          # [4, 64, S]
    kT2 = (
        kh.reshape(HPC, 64, KTP, 2, 128)
        .transpose(0, 3, 1, 2, 4)
        .reshape(HPC, 128, KTP * 128)
    )
    vv = v[b, h0 : h0 + HPC]                             # [4, S, 64]
    vh = np.concatenate(
        [vv, np.ones((HPC, S, 1), dtype=np.float32)], axis=2
    )                                                    # [4, S, 65]
    vh = vh.reshape(HPC, KT, 128, 65).transpose(0, 2, 1, 3).reshape(HPC, 128, KT * 65)
    mT = np.ascontiguousarray(mask[b, 0].T)              # [k, q]
    mF = (
        mT.reshape(KTP, 2, 128, QC, 512)
        .transpose(0, 2, 3, 1, 4)
        .reshape(KTP, 128, QC * 1024)
    )
    return {
        "qT2": np.ascontiguousarray(qT2).astype(BF16),
        "kT2": np.ascontiguousarray(kT2).astype(BF16),
        "vh": np.ascontiguousarray(vh).astype(BF16),
        "maskF": mF.astype(BF16),
    }


def kernel(q, k, v, mask):
    global _NC
    from concourse.bass_utils import run_bass_kernel_spmd

    q = np.asarray(q, dtype=np.float32)
    k = np.asarray(k, dtype=np.float32)
    v = np.asarray(v, dtype=np.float32)
    mask = np.asarray(mask)

    in_maps = [_core_inputs(q, k, v, mask, c) for c in range(NCORES)]
    if _NC is None:
        _NC = _build_nc()

    res = run_bass_kernel_spmd(
        _NC, in_maps, core_ids=list(range(NCORES)), trace=TRACE
    )
    LAST["exec_time_ns"] = res.exec_time_ns
    LAST["results"] = res

    out = np.empty((B, H, S, Dh), dtype=np.float32)
    for c in range(NCORES):
        b = c // HPC
        h0 = (c % HPC) * HPC
        o = res.results[c]["out"]                        # [4, 65, S] f32
        sums = o[:, Dh : Dh + 1, :]                      # [4, 1, S]
        on = o[:, :Dh, :] / sums                         # [4, 64, S]
        out[b, h0 : h0 + HPC] = on.transpose(0, 2, 1)
    return out


# revision 7
# speedup vs baseline: 1.0428x; 1.0428x over previous
"""Multi-head attention (B=2, H=16, S=2048, Dh=64) on 8 trn2 NeuronCores.

Sharding: core c handles batch c//4, heads (c%4)*4 .. +4. Each core computes
attention for its 4 (b,h) pairs independently (no collectives).

Device algorithm per (head, q-chunk of 512):
  S^T[k,q] = K @ Q^T   (bf16 matmuls, k-tile pairs row-packed into the
                        128-deep PE array since the contraction dim is 64)
  attE     = exp(0.125 * S^T)          (ScalarE, reads PSUM, writes bf16 SBUF)
  att      = attE * mask^T             (VectorE bf16; mask==0 rows -> 0)
  O^T[d,q] += [V | 1]^T @ att          (accumulated over k in PSUM; the ones
                                        column makes row 64 the softmax sums)
Host: normalize O^T rows by the sums row and transpose back to [S, Dh].
"""

import numpy as np
import ml_dtypes

TRACE = False
LAST = {}

B, H, S, Dh = 2, 16, 2048, 64
HPC = 4            # heads per core
NCORES = 8
SCALE = 0.125      # Dh ** -0.5
KT = 16            # k tiles of 128
KTP = 8            # k tile pairs
QC = 4             # q chunks of 512
BF16 = ml_dtypes.bfloat16

_NC = None


def _split_waits(nc, max_waits=1):
    """This container's walrus rejects >max_waits semaphore waits on one
    instruction (CoreV3 setupSyncWait "Too many sync wait commands"). Move
    the excess onto NoOps inserted just before, on the same engine — the
    per-engine instruction stream order is preserved, so the waits still
    complete before the original instruction issues."""
    import concourse.mybir as mybir

    ctr = 0
    for f in nc.m.functions:
        for b in f.blocks:
            insts = b.instructions
            new = []
            for inst in insts:
                si = inst.sync_info
                waits = list(si.on_wait) if si else []
                if len(waits) > max_waits:
                    for w in waits[:-max_waits]:
                        ctr += 1
                        new.append(
                            mybir.InstNoOp(
                                name=f"waitsplit-{ctr}",
                                engine=inst.engine,
                                ins=[],
                                outs=[],
                                sync_info=mybir.SyncInfo(on_wait=[w], on_update=[]),
                            )
                        )
                    inst.sync_info = mybir.SyncInfo(
                        on_wait=waits[-max_waits:], on_update=list(si.on_update)
                    )
                new.append(inst)
            insts[:] = new


def _build_nc():
    import concourse.bass as bass
    import concourse.mybir as mybir
    import concourse.tile as tile

    bf = mybir.dt.bfloat16
    f32 = mybir.dt.float32
    Exp = mybir.ActivationFunctionType.Exp

    nc = bass.Bass()
    qT2_e = nc.declare_dram_parameter("qT2", [HPC, 128, S], bf, isOutput=False)
    kT2_e = nc.declare_dram_parameter("kT2", [HPC, 128, KTP * 128], bf, isOutput=False)
    vh_e = nc.declare_dram_parameter("vh", [HPC, 128, KT * 65], bf, isOutput=False)
    mF_e = nc.declare_dram_parameter("maskF", [KTP, 128, QC * 1024], bf, isOutput=False)
    out_e = nc.declare_dram_parameter("out", [HPC, 65, S], f32, isOutput=True)

    with tile.TileContext(nc) as tc:
        with (
            tc.tile_pool(name="maskp", bufs=KTP) as maskp,
            tc.tile_pool(name="qp", bufs=2) as qp,
            tc.tile_pool(name="kp", bufs=2) as kp,
            tc.tile_pool(name="vp", bufs=2) as vp,
            tc.tile_pool(name="attp", bufs=4) as attp,
            tc.tile_pool(name="obp", bufs=2) as obp,
            tc.tile_pool(name="psp", bufs=3, space=bass.MemorySpace.PSUM) as psp,
            tc.tile_pool(name="pop", bufs=2, space=bass.MemorySpace.PSUM) as pop,
        ):
            # first head's tiles load BEFORE the 8.4MB of mask tiles so the
            # PE/ACT pipeline starts immediately; masks stream in behind
            qts, kts, vts = [None] * HPC, [None] * HPC, [None] * HPC
            qts[0] = qp.tile([128, S], bf, tag="q", name="qt0")
            nc.sync.dma_start(qts[0][:], qT2_e[0])
            kts[0] = kp.tile([128, KTP * 128], bf, tag="k", name="kt0")
            nc.sync.dma_start(kts[0][:], kT2_e[0])
            vts[0] = vp.tile([128, KT * 65], bf, tag="v", name="vt0")
            nc.sync.dma_start(vts[0][:], vh_e[0])

            mtiles = []
            for tp in range(KTP):
                mt = maskp.tile([128, QC * 1024], bf)
                nc.sync.dma_start(mt[:], mF_e[tp])
                mtiles.append(mt)

            for h in range(HPC):
                if qts[h] is None:
                    qts[h] = qp.tile([128, S], bf, tag="q", name=f"qt{h}")
                    nc.sync.dma_start(qts[h][:], qT2_e[h])
                    kts[h] = kp.tile([128, KTP * 128], bf, tag="k", name=f"kt{h}")
                    nc.sync.dma_start(kts[h][:], kT2_e[h])
                    vts[h] = vp.tile([128, KT * 65], bf, tag="v", name=f"vt{h}")
                    nc.sync.dma_start(vts[h][:], vh_e[h])
                qt, kt_, vt = qts[h], kts[h], vts[h]

                for qq in range(QC):
                    o_ps = pop.tile([65, 512], f32)
                    for tp in range(KTP):
                        s_ps = psp.tile([128, 1024], f32)
                        # S^T tiles for k-tiles 2tp (cols 0:512) and 2tp+1
                        # (cols 512:1024), row-packed in the PE array
                        nc.tensor.matmul(
                            s_ps[:, 0:512],
                            kt_[0:64, tp * 128 : (tp + 1) * 128],
                            qt[0:64, qq * 512 : (qq + 1) * 512],
                            start=True,
                            stop=True,
                        )
                        nc.tensor.matmul(
                            s_ps[:, 512:1024],
                            kt_[64:128, tp * 128 : (tp + 1) * 128],
                            qt[64:128, qq * 512 : (qq + 1) * 512],
                            start=True,
                            stop=True,
                        )
                        attE = attp.tile([128, 1024], bf, tag="attE")
                        nc.scalar.activation(attE[:], s_ps[:], Exp, scale=SCALE)
                        att = attp.tile([128, 1024], bf, tag="attm")
                        nc.vector.tensor_mul(
                            att[:],
                            attE[:],
                            mtiles[tp][:, qq * 1024 : (qq + 1) * 1024],
                        )
                        nc.tensor.matmul(
                            o_ps[:],
                            vt[:, (2 * tp) * 65 : (2 * tp + 1) * 65],
                            att[:, 0:512],
                            start=(tp == 0),
                            stop=False,
                        )
                        nc.tensor.matmul(
                            o_ps[:],
                            vt[:, (2 * tp + 1) * 65 : (2 * tp + 2) * 65],
                            att[:, 512:1024],
                            start=False,
                            stop=(tp == KTP - 1),
                        )
                    o_sb = obp.tile([65, 512], f32)
                    nc.vector.tensor_copy(o_sb[:], o_ps[:])
                    nc.sync.dma_start(
                        out_e[h, :, qq * 512 : (qq + 1) * 512], o_sb[:]
                    )
    _split_waits(nc)
    return nc


def _core_inputs(q, k, v, mask, core):
    b = core // HPC
    h0 = (core % HPC) * HPC
    qh = q[b, h0 : h0 + HPC].transpose(0, 2, 1)          # [4, 64, S]
    qT2 = np.concatenate([qh, qh], axis=1)               # [4, 128, S]
    kh = k[b, h0 : h0 + HPC].transpose(0, 2, 1)          # [4, 64, S]
    kT2 = (
        kh.reshape(HPC, 64, KTP, 2, 128)
        .transpose(0, 3, 1, 2, 4)
        .reshape(HPC, 128, KTP * 128)
    )
    vv = v[b, h0 : h0 + HPC]                             # [4, S, 64]
    vh = np.concatenate(
        [vv, np.ones((HPC, S, 1), dtype=np.float32)], axis=2
    )                                                    # [4, S, 65]
    vh = vh.reshape(HPC, KT, 128, 65).transpose(0, 2, 1, 3).reshape(HPC, 128, KT * 65)
    mT = np.ascontiguousarray(mask[b, 0].T)              # [k, q]
    mF = (
        mT.reshape(KTP, 2, 128, QC, 512)
        .transpose(0, 2, 3, 1, 4)
        .reshape(KTP, 128, QC * 1024)
    )
    return {
        "qT2": np.ascontiguousarray(qT2).astype(BF16),
        "kT2": np.ascontiguousarray(kT2).astype(BF16),
        "vh": np.ascontiguousarray(vh).astype(BF16),
        "maskF": mF.astype(BF16),
    }


def kernel(q, k, v, mask):
    global _NC
    from concourse.bass_utils import run_bass_kernel_spmd

    q = np.asarray(q, dtype=np.float32)
    k = np.asarray(k, dtype=np.float32)
    v = np.asarray(v, dtype=np.float32)
    mask = np.asarray(mask)

    in_maps = [_core_inputs(q, k, v, mask, c) for c in range(NCORES)]
    if _NC is None:
        _NC = _build_nc()

    res = run_bass_kernel_spmd(
        _NC, in_maps, core_ids=list(range(NCORES)), trace=TRACE
    )
    LAST["exec_time_ns"] = res.exec_time_ns
    LAST["results"] = res

    out = np.empty((B, H, S, Dh), dtype=np.float32)
    for c in range(NCORES):
        b = c // HPC
        h0 = (c % HPC) * HPC
        o = res.results[c]["out"]                        # [4, 65, S] f32
        sums = o[:, Dh : Dh + 1, :]                      # [4, 1, S]
        on = o[:, :Dh, :] / sums                         # [4, 64, S]
        out[b, h0 : h0 + HPC] = on.transpose(0, 2, 1)
    return out


# revision 10
# speedup vs baseline: 1.0495x; 1.0065x over previous
"""Multi-head attention (B=2, H=16, S=2048, Dh=64) on 8 trn2 NeuronCores.

Sharding: core c handles batch c//4, heads (c%4)*4 .. +4. Each core computes
attention for its 4 (b,h) pairs independently (no collectives).

Device algorithm per (head, q-chunk of 512):
  S^T[k,q] = K @ Q^T   (bf16 matmuls, k-tile pairs row-packed into the
                        128-deep PE array since the contraction dim is 64)
  attE     = exp(0.125 * S^T)          (ScalarE, reads PSUM, writes bf16 SBUF)
  att      = attE * mask^T             (VectorE bf16; mask==0 rows -> 0)
  O^T[d,q] += [V | 1]^T @ att          (accumulated over k in PSUM; the ones
                                        column makes row 64 the softmax sums)
Host: normalize O^T rows by the sums row and transpose back to [S, Dh].
"""

import numpy as np
import ml_dtypes

TRACE = False
LAST = {}

B, H, S, Dh = 2, 16, 2048, 64
HPC = 4            # heads per core
NCORES = 8
SCALE = 0.125      # Dh ** -0.5
KT = 16            # k tiles of 128
KTP = 8            # k tile pairs
QC = 4             # q chunks of 512
BF16 = ml_dtypes.bfloat16

_NC = None


def _split_waits(nc, max_waits=1):
    """This container's walrus rejects >max_waits semaphore waits on one
    instruction (CoreV3 setupSyncWait "Too many sync wait commands"). Move
    the excess onto NoOps inserted just before, on the same engine — the
    per-engine instruction stream order is preserved, so the waits still
    complete before the original instruction issues."""
    import concourse.mybir as mybir

    ctr = 0
    for f in nc.m.functions:
        for b in f.blocks:
            insts = b.instructions
            new = []
            for inst in insts:
                si = inst.sync_info
                waits = list(si.on_wait) if si else []
                if len(waits) > max_waits:
                    for w in waits[:-max_waits]:
                        ctr += 1
                        new.append(
                            mybir.InstNoOp(
                                name=f"waitsplit-{ctr}",
                                engine=inst.engine,
                                ins=[],
                                outs=[],
                                sync_info=mybir.SyncInfo(on_wait=[w], on_update=[]),
                            )
                        )
                    inst.sync_info = mybir.SyncInfo(
                        on_wait=waits[-max_waits:], on_update=list(si.on_update)
                    )
                new.append(inst)
            insts[:] = new


def _build_nc():
    import concourse.bass as bass
    import concourse.mybir as mybir
    import concourse.tile as tile

    bf = mybir.dt.bfloat16
    f32 = mybir.dt.float32
    Exp = mybir.ActivationFunctionType.Exp

    nc = bass.Bass()
    qT2_e = nc.declare_dram_parameter("qT2", [HPC, 128, S], bf, isOutput=False)
    kT2_e = nc.declare_dram_parameter("kT2", [HPC, 128, KTP * 128], bf, isOutput=False)
    vh_e = nc.declare_dram_parameter("vh", [HPC, 128, KT * 65], bf, isOutput=False)
    mF_e = nc.declare_dram_parameter("maskF", [KTP, 128, QC * 1024], bf, isOutput=False)
    out_e = nc.declare_dram_parameter("out", [HPC, 65, S], f32, isOutput=True)

    with tile.TileContext(nc) as tc:
        with (
            tc.tile_pool(name="maskp", bufs=KTP) as maskp,
            tc.tile_pool(name="qp", bufs=2) as qp,
            tc.tile_pool(name="kp", bufs=2) as kp,
            tc.tile_pool(name="vp", bufs=2) as vp,
            tc.tile_pool(name="attp", bufs=4) as attp,
            tc.tile_pool(name="obp", bufs=2) as obp,
            tc.tile_pool(name="wp", bufs=1) as wp,
            tc.tile_pool(name="psp", bufs=2, space=bass.MemorySpace.PSUM) as psp,
            tc.tile_pool(name="pop", bufs=1, space=bass.MemorySpace.PSUM) as pop,
        ):
            # touch the Exp table before any data arrives so the one-time
            # ACT_TABLE_LOAD overlaps the DMA ramp instead of the first tile
            warm_in = wp.tile([128, 8], bf, name="warm_in")
            nc.gpsimd.memset(warm_in[:], 0.0)
            warm_out = wp.tile([128, 8], bf, name="warm_out")
            nc.scalar.activation(warm_out[:], warm_in[:], Exp, scale=1.0)

            # first head's tiles load BEFORE the 8.4MB of mask tiles so the
            # PE/ACT pipeline starts immediately; masks stream in behind
            qts, kts, vts = [None] * HPC, [None] * HPC, [None] * HPC
            qts[0] = qp.tile([128, S], bf, tag="q", name="qt0")
            nc.sync.dma_start(qts[0][:], qT2_e[0])
            kts[0] = kp.tile([128, KTP * 128], bf, tag="k", name="kt0")
            nc.sync.dma_start(kts[0][:], kT2_e[0])
            vts[0] = vp.tile([128, KT * 65], bf, tag="v", name="vt0")
            nc.sync.dma_start(vts[0][:], vh_e[0])

            mtiles = []
            for tp in range(KTP):
                mt = maskp.tile([128, QC * 1024], bf)
                nc.sync.dma_start(mt[:], mF_e[tp])
                mtiles.append(mt)

            for h in range(HPC):
                if qts[h] is None:
                    qts[h] = qp.tile([128, S], bf, tag="q", name=f"qt{h}")
                    nc.sync.dma_start(qts[h][:], qT2_e[h])
                    kts[h] = kp.tile([128, KTP * 128], bf, tag="k", name=f"kt{h}")
                    nc.sync.dma_start(kts[h][:], kT2_e[h])
                    vts[h] = vp.tile([128, KT * 65], bf, tag="v", name=f"vt{h}")
                    nc.sync.dma_start(vts[h][:], vh_e[h])
                qt, kt_, vt = qts[h], kts[h], vts[h]

                # tp outer / qq inner: each 1MB mask tile is consumed over 4
                # iterations, so the mask DMA stream stays ahead of the
                # pipeline from the very first head. The 4 per-q-chunk O
                # accumulators live in PSUM simultaneously (4 banks).
                o_list = [
                    pop.tile([65, 512], f32, name=f"o_{h}_{qq}", tag=f"o{qq}")
                    for qq in range(QC)
                ]
                for tp in range(KTP):
                    for qq in range(QC):
                        s_ps = psp.tile(
                            [128, 1024], f32, name=f"s_{h}_{tp}_{qq}", tag="s"
                        )
                        # S^T tiles for k-tiles 2tp (cols 0:512) and 2tp+1
                        # (cols 512:1024), row-packed in the PE array
                        nc.tensor.matmul(
                            s_ps[:, 0:512],
                            kt_[0:64, tp * 128 : (tp + 1) * 128],
                            qt[0:64, qq * 512 : (qq + 1) * 512],
                            start=True,
                            stop=True,
                        )
                        nc.tensor.matmul(
                            s_ps[:, 512:1024],
                            kt_[64:128, tp * 128 : (tp + 1) * 128],
                            qt[64:128, qq * 512 : (qq + 1) * 512],
                            start=True,
                            stop=True,
                        )
                        attE = attp.tile([128, 1024], bf, tag="attE", name="attE")
                        nc.scalar.activation(attE[:], s_ps[:], Exp, scale=SCALE)
                        att = attp.tile([128, 1024], bf, tag="attm", name="attm")
                        nc.vector.tensor_mul(
                            att[:],
                            attE[:],
                            mtiles[tp][:, qq * 1024 : (qq + 1) * 1024],
                        )
                        nc.tensor.matmul(
                            o_list[qq][:],
                            vt[:, (2 * tp) * 65 : (2 * tp + 1) * 65],
                            att[:, 0:512],
                            start=(tp == 0),
                            stop=False,
                        )
                        nc.tensor.matmul(
                            o_list[qq][:],
                            vt[:, (2 * tp + 1) * 65 : (2 * tp + 2) * 65],
                            att[:, 512:1024],
                            start=False,
                            stop=(tp == KTP - 1),
                        )
                for qq in range(QC):
                    o_sb = obp.tile([65, 512], f32, name=f"osb_{h}_{qq}", tag="osb")
                    nc.vector.tensor_copy(o_sb[:], o_list[qq][:])
                    nc.sync.dma_start(
                        out_e[h, :, qq * 512 : (qq + 1) * 512], o_sb[:]
                    )
    _split_waits(nc)
    return nc


def _core_inputs(q, k, v, mask, core):
    b = core // HPC
    h0 = (core % HPC) * HPC
    qh = q[b, h0 : h0 + HPC].transpose(0, 2, 1)          # [4, 64, S]
    qT2 = np.concatenate([qh, qh], axis=1)               # [4, 128, S]
    kh = k[b, h0 : h0 + HPC].transpose(0, 2, 1)          # [4, 64, S]
    kT2 = (
        kh.reshape(HPC, 64, KTP, 2, 128)
        .transpose(0, 3, 1, 2, 4)
        .reshape(HPC, 128, KTP * 128)
    )
    vv = v[b, h0 : h0 + HPC]                             # [4, S, 64]
    vh = np.concatenate(
        [vv, np.ones((HPC, S, 1), dtype=np.float32)], axis=2
    )                                                    # [4, S, 65]
    vh = vh.reshape(HPC, KT, 128, 65).transpose(0, 2, 1, 3).reshape(HPC, 128, KT * 65)
    mT = np.ascontiguousarray(mask[b, 0].T)              # [k, q]
    mF = (
        mT.reshape(KTP, 2, 128, QC, 512)
        .transpose(0, 2, 3, 1, 4)
        .reshape(KTP, 128, QC * 1024)
    )
    return {
        "qT2": np.ascontiguousarray(qT2).astype(BF16),
        "kT2": np.ascontiguousarray(kT2).astype(BF16),
        "vh": np.ascontiguousarray(vh).astype(BF16),
        "maskF": mF.astype(BF16),
    }


def kernel(q, k, v, mask):
    global _NC
    from concourse.bass_utils import run_bass_kernel_spmd

    q = np.asarray(q, dtype=np.float32)
    k = np.asarray(k, dtype=np.float32)
    v = np.asarray(v, dtype=np.float32)
    mask = np.asarray(mask)

    in_maps = [_core_inputs(q, k, v, mask, c) for c in range(NCORES)]
    if _NC is None:
        _NC = _build_nc()

    res = run_bass_kernel_spmd(
        _NC, in_maps, core_ids=list(range(NCORES)), trace=TRACE
    )
    LAST["exec_time_ns"] = res.exec_time_ns
    LAST["results"] = res

    out = np.empty((B, H, S, Dh), dtype=np.float32)
    for c in range(NCORES):
        b = c // HPC
        h0 = (c % HPC) * HPC
        o = res.results[c]["out"]                        # [4, 65, S] f32
        sums = o[:, Dh : Dh + 1, :]                      # [4, 1, S]
        on = o[:, :Dh, :] / sums                         # [4, 64, S]
        out[b, h0 : h0 + HPC] = on.transpose(0, 2, 1)
    return out


# revision 13
# speedup vs baseline: 1.0576x; 1.0077x over previous
"""Multi-head attention (B=2, H=16, S=2048, Dh=64) on 8 trn2 NeuronCores.

Sharding: core c handles batch c//4, heads (c%4)*4 .. +4. Each core computes
attention for its 4 (b,h) pairs independently (no collectives).

Device algorithm per (head, q-chunk of 512):
  S^T[k,q] = K @ Q^T   (bf16 matmuls, k-tile pairs row-packed into the
                        128-deep PE array since the contraction dim is 64)
  attE     = exp(0.125 * S^T)          (ScalarE, reads PSUM, writes bf16 SBUF)
  att      = attE * mask^T             (VectorE bf16; mask==0 rows -> 0)
  O^T[d,q] += [V | 1]^T @ att          (accumulated over k in PSUM; the ones
                                        column makes row 64 the softmax sums)
Host: normalize O^T rows by the sums row and transpose back to [S, Dh].
"""

import numpy as np
import ml_dtypes

TRACE = False
LAST = {}

B, H, S, Dh = 2, 16, 2048, 64
HPC = 4            # heads per core
NCORES = 8
SCALE = 0.125      # Dh ** -0.5
KT = 16            # k tiles of 128
KTP = 8            # k tile pairs
QC = 4             # q chunks of 512
BF16 = ml_dtypes.bfloat16

_NC = None


def _split_waits(nc, max_waits=1):
    """This container's walrus rejects >max_waits semaphore waits on one
    instruction (CoreV3 setupSyncWait "Too many sync wait commands"). Move
    the excess onto NoOps inserted just before, on the same engine — the
    per-engine instruction stream order is preserved, so the waits still
    complete before the original instruction issues."""
    import concourse.mybir as mybir

    ctr = 0
    for f in nc.m.functions:
        for b in f.blocks:
            insts = b.instructions
            new = []
            for inst in insts:
                si = inst.sync_info
                waits = list(si.on_wait) if si else []
                if len(waits) > max_waits:
                    for w in waits[:-max_waits]:
                        ctr += 1
                        new.append(
                            mybir.InstNoOp(
                                name=f"waitsplit-{ctr}",
                                engine=inst.engine,
                                ins=[],
                                outs=[],
                                sync_info=mybir.SyncInfo(on_wait=[w], on_update=[]),
                            )
                        )
                    inst.sync_info = mybir.SyncInfo(
                        on_wait=waits[-max_waits:], on_update=list(si.on_update)
                    )
                new.append(inst)
            insts[:] = new


def _patch_minimal_teardown():
    """Tile's exit emits drain + 2 all-engine barriers + semaphore clears
    (~10us on the critical tail). The barriers/clears only matter for
    re-executing an already-loaded NEFF; each kernel() call loads fresh, so
    keep just the final drain (it carries the waits that guarantee the
    output DMAs completed)."""
    import concourse.tile as tile
    from concourse.vector_clock import ScopedClock

    if getattr(tile.TileContext._drain_and_barrier, "_minimal", False):
        return

    def _drain_and_barrier(self, tick_clock, wait_clock):
        drain_inst = self.nc.sync.drain()
        wait_clock.add_sem_waits(
            drain_inst.ins, ScopedClock({None: tick_clock.global_clock})
        )
        popped = self.nc._tile_sem_poison_stack.pop()
        assert popped is self._sem_poison

    _drain_and_barrier._minimal = True
    tile.TileContext._drain_and_barrier = _drain_and_barrier


def _build_nc():
    import concourse.bass as bass
    import concourse.mybir as mybir
    import concourse.tile as tile

    _patch_minimal_teardown()
    bf = mybir.dt.bfloat16
    f32 = mybir.dt.float32
    Exp = mybir.ActivationFunctionType.Exp

    nc = bass.Bass()
    qT2_e = nc.declare_dram_parameter("qT2", [HPC, 128, S], bf, isOutput=False)
    kT2_e = nc.declare_dram_parameter("kT2", [HPC, 128, KTP * 128], bf, isOutput=False)
    vh_e = nc.declare_dram_parameter("vh", [HPC, 128, KT * 65], bf, isOutput=False)
    mF_e = nc.declare_dram_parameter("maskF", [KTP, 128, QC * 1024], bf, isOutput=False)
    out_e = nc.declare_dram_parameter("out", [HPC, 65, S], f32, isOutput=True)

    with tile.TileContext(nc) as tc:
        with (
            tc.tile_pool(name="maskp", bufs=KTP) as maskp,
            tc.tile_pool(name="qp", bufs=2) as qp,
            tc.tile_pool(name="kp", bufs=2) as kp,
            tc.tile_pool(name="vp", bufs=2) as vp,
            tc.tile_pool(name="attp", bufs=4) as attp,
            tc.tile_pool(name="obp", bufs=2) as obp,
            tc.tile_pool(name="wp", bufs=1) as wp,
            tc.tile_pool(name="psp", bufs=2, space=bass.MemorySpace.PSUM) as psp,
            tc.tile_pool(name="pop", bufs=1, space=bass.MemorySpace.PSUM) as pop,
        ):
            # touch the Exp table before any data arrives so the one-time
            # ACT_TABLE_LOAD overlaps the DMA ramp instead of the first tile
            warm_in = wp.tile([128, 8], bf, name="warm_in")
            nc.gpsimd.memset(warm_in[:], 0.0)
            warm_out = wp.tile([128, 8], bf, name="warm_out")
            nc.scalar.activation(warm_out[:], warm_in[:], Exp, scale=1.0)

            # first head's tiles load BEFORE the 8.4MB of mask tiles so the
            # PE/ACT pipeline starts immediately; masks stream in behind
            qts, kts, vts = [None] * HPC, [None] * HPC, [None] * HPC
            qts[0] = qp.tile([128, S], bf, tag="q", name="qt0")
            nc.sync.dma_start(qts[0][:], qT2_e[0])
            kts[0] = kp.tile([128, KTP * 128], bf, tag="k", name="kt0")
            nc.sync.dma_start(kts[0][:], kT2_e[0])
            vts[0] = vp.tile([128, KT * 65], bf, tag="v", name="vt0")
            nc.sync.dma_start(vts[0][:], vh_e[0])

            mtiles = []
            for tp in range(KTP):
                mt = maskp.tile([128, QC * 1024], bf)
                nc.sync.dma_start(mt[:], mF_e[tp])
                mtiles.append(mt)

            for h in range(HPC):
                if qts[h] is None:
                    qts[h] = qp.tile([128, S], bf, tag="q", name=f"qt{h}")
                    nc.sync.dma_start(qts[h][:], qT2_e[h])
                    kts[h] = kp.tile([128, KTP * 128], bf, tag="k", name=f"kt{h}")
                    nc.sync.dma_start(kts[h][:], kT2_e[h])
                    vts[h] = vp.tile([128, KT * 65], bf, tag="v", name=f"vt{h}")
                    nc.sync.dma_start(vts[h][:], vh_e[h])
                qt, kt_, vt = qts[h], kts[h], vts[h]

                # tp outer / qq inner: each 1MB mask tile is consumed over 4
                # iterations, so the mask DMA stream stays ahead of the
                # pipeline from the very first head. The 4 per-q-chunk O
                # accumulators live in PSUM simultaneously (4 banks).
                o_list = [
                    pop.tile([65, 512], f32, name=f"o_{h}_{qq}", tag=f"o{qq}")
                    for qq in range(QC)
                ]
                # last head runs qq-outer so each o[qq] finishes (and its
                # drain-copy + store DMA overlap compute) before the kernel
                # tail; earlier heads run tp-outer so the mask DMA stream
                # stays ahead of the pipeline during the ramp
                last = h == HPC - 1
                if last:
                    order = [(tp, qq) for qq in range(QC) for tp in range(KTP)]
                else:
                    order = [(tp, qq) for tp in range(KTP) for qq in range(QC)]
                for tp, qq in order:
                    if True:
                        s_ps = psp.tile(
                            [128, 1024], f32, name=f"s_{h}_{tp}_{qq}", tag="s"
                        )
                        # S^T tiles for k-tiles 2tp (cols 0:512) and 2tp+1
                        # (cols 512:1024), row-packed in the PE array
                        nc.tensor.matmul(
                            s_ps[:, 0:512],
                            kt_[0:64, tp * 128 : (tp + 1) * 128],
                            qt[0:64, qq * 512 : (qq + 1) * 512],
                            start=True,
                            stop=True,
                        )
                        nc.tensor.matmul(
                            s_ps[:, 512:1024],
                            kt_[64:128, tp * 128 : (tp + 1) * 128],
                            qt[64:128, qq * 512 : (qq + 1) * 512],
                            start=True,
                            stop=True,
                        )
                        attE = attp.tile([128, 1024], bf, tag="attE", name="attE")
                        nc.scalar.activation(attE[:], s_ps[:], Exp, scale=SCALE)
                        att = attp.tile([128, 1024], bf, tag="attm", name="attm")
                        nc.vector.tensor_mul(
                            att[:],
                            attE[:],
                            mtiles[tp][:, qq * 1024 : (qq + 1) * 1024],
                        )
                        nc.tensor.matmul(
                            o_list[qq][:],
                            vt[:, (2 * tp) * 65 : (2 * tp + 1) * 65],
                            att[:, 0:512],
                            start=(tp == 0),
                            stop=False,
                        )
                        nc.tensor.matmul(
                            o_list[qq][:],
                            vt[:, (2 * tp + 1) * 65 : (2 * tp + 2) * 65],
                            att[:, 512:1024],
                            start=False,
                            stop=(tp == KTP - 1),
                        )
                        if tp == KTP - 1:
                            o_sb = obp.tile(
                                [65, 512], f32, name=f"osb_{h}_{qq}", tag="osb"
                            )
                            nc.vector.tensor_copy(o_sb[:], o_list[qq][:])
                            nc.sync.dma_start(
                                out_e[h, :, qq * 512 : (qq + 1) * 512], o_sb[:]
                            )
    _split_waits(nc)
    return nc


def _core_inputs(q, k, v, mask, core):
    b = core // HPC
    h0 = (core % HPC) * HPC
    qh = q[b, h0 : h0 + HPC].transpose(0, 2, 1)          # [4, 64, S]
    qT2 = np.concatenate([qh, qh], axis=1)               # [4, 128, S]
    kh = k[b, h0 : h0 + HPC].transpose(0, 2, 1)          # [4, 64, S]
    kT2 = (
        kh.reshape(HPC, 64, KTP, 2, 128)
        .transpose(0, 3, 1, 2, 4)
        .reshape(HPC, 128, KTP * 128)
    )
    vv = v[b, h0 : h0 + HPC]                             # [4, S, 64]
    vh = np.concatenate(
        [vv, np.ones((HPC, S, 1), dtype=np.float32)], axis=2
    )                                                    # [4, S, 65]
    vh = vh.reshape(HPC, KT, 128, 65).transpose(0, 2, 1, 3).reshape(HPC, 128, KT * 65)
    mT = np.ascontiguousarray(mask[b, 0].T)              # [k, q]
    mF = (
        mT.reshape(KTP, 2, 128, QC, 512)
        .transpose(0, 2, 3, 1, 4)
        .reshape(KTP, 128, QC * 1024)
    )
    return {
        "qT2": np.ascontiguousarray(qT2).astype(BF16),
        "kT2": np.ascontiguousarray(kT2).astype(BF16),
        "vh": np.ascontiguousarray(vh).astype(BF16),
        "maskF": mF.astype(BF16),
    }


def kernel(q, k, v, mask):
    global _NC
    from concourse.bass_utils import run_bass_kernel_spmd

    q = np.asarray(q, dtype=np.float32)
    k = np.asarray(k, dtype=np.float32)
    v = np.asarray(v, dtype=np.float32)
    mask = np.asarray(mask)

    in_maps = [_core_inputs(q, k, v, mask, c) for c in range(NCORES)]
    if _NC is None:
        _NC = _build_nc()

    res = run_bass_kernel_spmd(
        _NC, in_maps, core_ids=list(range(NCORES)), trace=TRACE
    )
    LAST["exec_time_ns"] = res.exec_time_ns
    LAST["results"] = res

    out = np.empty((B, H, S, Dh), dtype=np.float32)
    for c in range(NCORES):
        b = c // HPC
        h0 = (c % HPC) * HPC
        o = res.results[c]["out"]                        # [4, 65, S] f32
        sums = o[:, Dh : Dh + 1, :]                      # [4, 1, S]
        on = o[:, :Dh, :] / sums                         # [4, 64, S]
        out[b, h0 : h0 + HPC] = on.transpose(0, 2, 1)
    return out


# revision 15
# speedup vs baseline: 1.0592x; 1.0014x over previous
"""Multi-head attention (B=2, H=16, S=2048, Dh=64) on 8 trn2 NeuronCores.

Sharding: core c handles batch c//4, heads (c%4)*4 .. +4. Each core computes
attention for its 4 (b,h) pairs independently (no collectives).

Device algorithm per (head, q-chunk of 512):
  S^T[k,q] = K @ Q^T   (bf16 matmuls, k-tile pairs row-packed into the
                        128-deep PE array since the contraction dim is 64)
  attE     = exp(0.125 * S^T)          (ScalarE, reads PSUM, writes bf16 SBUF)
  att      = attE * mask^T             (VectorE bf16; mask==0 rows -> 0)
  O^T[d,q] += [V | 1]^T @ att          (accumulated over k in PSUM; the ones
                                        column makes row 64 the softmax sums)
Host: normalize O^T rows by the sums row and transpose back to [S, Dh].
"""

import numpy as np
import ml_dtypes

TRACE = False
LAST = {}

B, H, S, Dh = 2, 16, 2048, 64
HPC = 4            # heads per core
NCORES = 8
SCALE = 0.125      # Dh ** -0.5
KT = 16            # k tiles of 128
KTP = 8            # k tile pairs
QC = 4             # q chunks of 512
BF16 = ml_dtypes.bfloat16

_NC = None


def _split_waits(nc, max_waits=1):
    """This container's walrus rejects >max_waits semaphore waits on one
    instruction (CoreV3 setupSyncWait "Too many sync wait commands"). Move
    the excess onto NoOps inserted just before, on the same engine — the
    per-engine instruction stream order is preserved, so the waits still
    complete before the original instruction issues."""
    import concourse.mybir as mybir

    ctr = 0
    for f in nc.m.functions:
        for b in f.blocks:
            insts = b.instructions
            new = []
            for inst in insts:
                si = inst.sync_info
                waits = list(si.on_wait) if si else []
                if len(waits) > max_waits:
                    for w in waits[:-max_waits]:
                        ctr += 1
                        new.append(
                            mybir.InstNoOp(
                                name=f"waitsplit-{ctr}",
                                engine=inst.engine,
                                ins=[],
                                outs=[],
                                sync_info=mybir.SyncInfo(on_wait=[w], on_update=[]),
                            )
                        )
                    inst.sync_info = mybir.SyncInfo(
                        on_wait=waits[-max_waits:], on_update=list(si.on_update)
                    )
                new.append(inst)
            insts[:] = new


def _patch_minimal_teardown():
    """Tile's exit emits drain + 2 all-engine barriers + semaphore clears
    (~10us on the critical tail). The barriers/clears only matter for
    re-executing an already-loaded NEFF; each kernel() call loads fresh, so
    keep just the final drain (it carries the waits that guarantee the
    output DMAs completed)."""
    import concourse.tile as tile
    from concourse.vector_clock import ScopedClock

    if getattr(tile.TileContext._drain_and_barrier, "_minimal", False):
        return

    def _drain_and_barrier(self, tick_clock, wait_clock):
        drain_inst = self.nc.sync.drain()
        wait_clock.add_sem_waits(
            drain_inst.ins, ScopedClock({None: tick_clock.global_clock})
        )
        popped = self.nc._tile_sem_poison_stack.pop()
        assert popped is self._sem_poison

    _drain_and_barrier._minimal = True
    tile.TileContext._drain_and_barrier = _drain_and_barrier


def _build_nc():
    import concourse.bass as bass
    import concourse.mybir as mybir
    import concourse.tile as tile

    _patch_minimal_teardown()
    bf = mybir.dt.bfloat16
    f32 = mybir.dt.float32
    Exp = mybir.ActivationFunctionType.Exp

    nc = bass.Bass()
    qT2_e = nc.declare_dram_parameter("qT2", [HPC, 128, S], bf, isOutput=False)
    kT2_e = nc.declare_dram_parameter("kT2", [HPC, 128, KTP * 128], bf, isOutput=False)
    vh_e = nc.declare_dram_parameter("vh", [HPC, 128, KT * 65], bf, isOutput=False)
    mF_e = nc.declare_dram_parameter("maskF", [KTP, 128, QC * 1024], bf, isOutput=False)
    out_e = nc.declare_dram_parameter("out", [HPC, 65, S], f32, isOutput=True)

    with tile.TileContext(nc) as tc:
        with (
            tc.tile_pool(name="maskp", bufs=KTP) as maskp,
            tc.tile_pool(name="qp", bufs=2) as qp,
            tc.tile_pool(name="kp", bufs=2) as kp,
            tc.tile_pool(name="vp", bufs=2) as vp,
            tc.tile_pool(name="attp", bufs=4) as attp,
            tc.tile_pool(name="obp", bufs=2) as obp,
            tc.tile_pool(name="wp", bufs=1) as wp,
            tc.tile_pool(name="psp", bufs=2, space=bass.MemorySpace.PSUM) as psp,
            tc.tile_pool(name="pop", bufs=1, space=bass.MemorySpace.PSUM) as pop,
        ):
            # touch the Exp table before any data arrives so the one-time
            # ACT_TABLE_LOAD overlaps the DMA ramp instead of the first tile
            warm_in = wp.tile([128, 8], bf, name="warm_in")
            nc.gpsimd.memset(warm_in[:], 0.0)
            warm_out = wp.tile([128, 8], bf, name="warm_out")
            nc.scalar.activation(warm_out[:], warm_in[:], Exp, scale=1.0)

            # first head's tiles load BEFORE the 8.4MB of mask tiles so the
            # PE/ACT pipeline starts immediately; masks stream in behind
            qts, kts, vts = [None] * HPC, [None] * HPC, [None] * HPC
            qts[0] = qp.tile([128, S], bf, tag="q", name="qt0")
            nc.sync.dma_start(qts[0][:], qT2_e[0])
            kts[0] = kp.tile([128, KTP * 128], bf, tag="k", name="kt0")
            nc.sync.dma_start(kts[0][:], kT2_e[0])
            vts[0] = vp.tile([128, KT * 65], bf, tag="v", name="vt0")
            nc.sync.dma_start(vts[0][:], vh_e[0])

            mtiles = []
            for tp in range(KTP):
                mt = maskp.tile([128, QC * 1024], bf)
                nc.sync.dma_start(mt[:], mF_e[tp])
                mtiles.append(mt)

            for h in range(HPC):
                if qts[h] is None:
                    qts[h] = qp.tile([128, S], bf, tag="q", name=f"qt{h}")
                    nc.sync.dma_start(qts[h][:], qT2_e[h])
                    kts[h] = kp.tile([128, KTP * 128], bf, tag="k", name=f"kt{h}")
                    nc.sync.dma_start(kts[h][:], kT2_e[h])
                    vts[h] = vp.tile([128, KT * 65], bf, tag="v", name=f"vt{h}")
                    nc.sync.dma_start(vts[h][:], vh_e[h])
                qt, kt_, vt = qts[h], kts[h], vts[h]

                # tp outer / qq inner: each 1MB mask tile is consumed over 4
                # iterations, so the mask DMA stream stays ahead of the
                # pipeline from the very first head. The 4 per-q-chunk O
                # accumulators live in PSUM simultaneously (4 banks).
                o_list = [
                    pop.tile([65, 512], f32, name=f"o_{h}_{qq}", tag=f"o{qq}")
                    for qq in range(QC)
                ]
                # last head runs qq-outer so each o[qq] finishes (and its
                # drain-copy + store DMA overlap compute) before the kernel
                # tail; earlier heads run tp-outer so the mask DMA stream
                # stays ahead of the pipeline during the ramp
                last = h == HPC - 1
                if last:
                    order = [(tp, qq) for qq in range(QC) for tp in range(KTP)]
                else:
                    order = [(tp, qq) for tp in range(KTP) for qq in range(QC)]
                for tp, qq in order:
                    if True:
                        s_ps = psp.tile(
                            [128, 1024], f32, name=f"s_{h}_{tp}_{qq}", tag="s"
                        )
                        # S^T tiles for k-tiles 2tp (cols 0:512) and 2tp+1
                        # (cols 512:1024), row-packed in the PE array
                        nc.tensor.matmul(
                            s_ps[:, 0:512],
                            kt_[0:64, tp * 128 : (tp + 1) * 128],
                            qt[0:64, qq * 512 : (qq + 1) * 512],
                            start=True,
                            stop=True,
                        )
                        nc.tensor.matmul(
                            s_ps[:, 512:1024],
                            kt_[64:128, tp * 128 : (tp + 1) * 128],
                            qt[64:128, qq * 512 : (qq + 1) * 512],
                            start=True,
                            stop=True,
                        )
                        attE = attp.tile([128, 1024], bf, tag="attE", name="attE")
                        nc.scalar.activation(attE[:], s_ps[:], Exp, scale=SCALE)
                        att = attp.tile([128, 1024], bf, tag="attm", name="attm")
                        nc.vector.tensor_mul(
                            att[:],
                            attE[:],
                            mtiles[tp][:, qq * 1024 : (qq + 1) * 1024],
                        )
                        nc.tensor.matmul(
                            o_list[qq][:],
                            vt[:, (2 * tp) * 65 : (2 * tp + 1) * 65],
                            att[:, 0:512],
                            start=(tp == 0),
                            stop=False,
                        )
                        nc.tensor.matmul(
                            o_list[qq][:],
                            vt[:, (2 * tp + 1) * 65 : (2 * tp + 2) * 65],
                            att[:, 512:1024],
                            start=False,
                            stop=(tp == KTP - 1),
                        )
                        if tp == KTP - 1:
                            o_sb = obp.tile(
                                [65, 512], f32, name=f"osb_{h}_{qq}", tag="osb"
                            )
                            nc.vector.tensor_copy(o_sb[:], o_list[qq][:])
                            nc.sync.dma_start(
                                out_e[h, :, qq * 512 : (qq + 1) * 512], o_sb[:]
                            )
    _split_waits(nc)
    return nc


def _core_inputs(q, k, v, mask, core):
    b = core // HPC
    h0 = (core % HPC) * HPC
    qh = q[b, h0 : h0 + HPC].transpose(0, 2, 1)          # [4, 64, S]
    qT2 = np.concatenate([qh, qh], axis=1)               # [4, 128, S]
    kh = k[b, h0 : h0 + HPC].transpose(0, 2, 1)          # [4, 64, S]
    kT2 = (
        kh.reshape(HPC, 64, KTP, 2, 128)
        .transpose(0, 3, 1, 2, 4)
        .reshape(HPC, 128, KTP * 128)
    )
    vv = v[b, h0 : h0 + HPC]                             # [4, S, 64]
    vh = np.concatenate(
        [vv, np.ones((HPC, S, 1), dtype=np.float32)], axis=2
    )                                                    # [4, S, 65]
    vh = vh.reshape(HPC, KT, 128, 65).transpose(0, 2, 1, 3).reshape(HPC, 128, KT * 65)
    mT = np.ascontiguousarray(mask[b, 0].T)              # [k, q]
    mF = (
        mT.reshape(KTP, 2, 128, QC, 512)
        .transpose(0, 2, 3, 1, 4)
        .reshape(KTP, 128, QC * 1024)
    )
    return {
        "qT2": np.ascontiguousarray(qT2).astype(BF16),
        "kT2": np.ascontiguousarray(kT2).astype(BF16),
        "vh": np.ascontiguousarray(vh).astype(BF16),
        "maskF": mF.astype(BF16),
    }


def kernel(q, k, v, mask):
    global _NC
    from concourse.bass_utils import run_bass_kernel_spmd

    q = np.asarray(q, dtype=np.float32)
    k = np.asarray(k, dtype=np.float32)
    v = np.asarray(v, dtype=np.float32)
    mask = np.asarray(mask)

    in_maps = [_core_inputs(q, k, v, mask, c) for c in range(NCORES)]
    if _NC is None:
        _NC = _build_nc()

    res = run_bass_kernel_spmd(
        _NC, in_maps, core_ids=list(range(NCORES)), trace=TRACE
    )
    LAST["exec_time_ns"] = res.exec_time_ns
    LAST["results"] = res

    out = np.empty((B, H, S, Dh), dtype=np.float32)
    for c in range(NCORES):
        b = c // HPC
        h0 = (c % HPC) * HPC
        o = res.results[c]["out"]                        # [4, 65, S] f32
        sums = o[:, Dh : Dh + 1, :]                      # [4, 1, S]
        on = o[:, :Dh, :] / sums                         # [4, 64, S]
        out[b, h0 : h0 + HPC] = on.transpose(0, 2, 1)
    return out
